# revision 1
# baseline (speedup 1.0000x reference)
"""Trainium2 Bass kernel for nn_NextRowPredictionHead (loss_fn).

Sharding: feature-parallel across 8 cores. Each core gets 4 categorical +
2 numerical + 2 boolean features over the FULL batch (B=2048), so every
per-feature masked-mean reduction is core-local; the host just sums the
8 partial (sum, count) vectors and does the final divides.

Device dataflow (per core, per feature f, per batch-tile of 512):
  - activations live transposed [feature_dim on partitions, batch on free]
  - shared:  h_pre^T = W1^T-matmul(x^T), GELU+b1 via ACT (bias is
    per-partition in this layout)
  - LayerNorm stats over the partition dim via PE ones-reduce matmuls;
    the per-column (per-sample) affine is NOT applied to the activations.
    Instead mean/std enter the next matmul as rank-1 K=1 rows
    (float32r, so no cast needed) and the positive scale 1/std rides
    through relu and is applied inside the ACT epilogue of the second
    head matmul (scale= per-partition AP, natural layout there).
  - head2 outputs logits in natural layout [batch on partitions], so
    softmax reductions are free-dim ops; the picked logit for CE comes
    from host-pre-gathered Wc2[:, target] columns via an elementwise
    product + ones-reduce.
"""

import sys
from contextlib import ExitStack

import numpy as np

sys.path.insert(0, "/opt/trn_rl_repo")

import concourse.bass as bass  # noqa: E402
import concourse.tile as tile  # noqa: E402
from concourse import bacc, mybir  # noqa: E402
from concourse.bass_utils import run_bass_kernel_spmd  # noqa: E402

F32 = mybir.dt.float32
BF16 = mybir.dt.bfloat16
F32R = mybir.dt.float32r
AF = mybir.ActivationFunctionType
OP = mybir.AluOpType

P = 128
D = 512
H = 256
V = 1000
B = 2048
NC, NN, NB = 32, 16, 16
FC, FN, FB = 4, 2, 2      # per-core feature counts
NF = FC + FN + FB         # 8 local features
NBT = 4                   # batch tiles of 512
BT = 512
NBS = 4                   # 128-row subtiles per batch tile
NS = NBT * NBS            # 16 subtiles of 128
LN_EPS = 1e-5
N_CORES = 8

LAST_RESULTS = None


def _build():
    nc = bacc.Bacc("TRN2", target_bir_lowering=False, debug=False,
                   num_devices=N_CORES)

    io = {}

    def din(name, shape):
        io[name] = nc.dram_tensor(name, shape, F32, kind="ExternalInput").ap()

    din("xt", [NF, D, B])
    din("w1", [D, D])
    din("b1", [D])
    din("wc1", [FC, D, H])
    din("wc1r", [FC, 2, H])
    din("wc2", [FC, H, V])
    din("bc2", [FC, V])
    din("wct", [FC, H, B])
    din("bc2t", [FC, NS, P])
    din("maskc", [FC, NS, P])
    din("wn1", [FN, D, H])
    din("wn1r", [FN, 2, H])
    din("wn2", [FN, H, D])
    din("bn2", [FN, D])
    din("tgt", [FN, B, D])
    din("maskn", [FN, NS, P])
    din("wb1", [FB, D, H])
    din("wb1r", [FB, 2, H])
    din("wb2", [FB, H, 2])
    din("bb2", [FB, 2])
    din("boolt", [FB, NS, P])
    din("maskb", [FB, NS, P])
    out = nc.dram_tensor("loss_out", [2 * NF], F32, kind="ExternalOutput").ap()

    with tile.TileContext(nc) as tc:
        with ExitStack() as ctx:
            build_body(ctx, tc, io, out)
    nc.compile()
    return nc


def build_body(ctx, tc, io, out):
    nc = tc.nc

    const = ctx.enter_context(tc.tile_pool(name="const", bufs=1))
    wpool = ctx.enter_context(tc.tile_pool(name="wpool", bufs=2))
    percf = ctx.enter_context(tc.tile_pool(name="percf", bufs=2))
    xpool = ctx.enter_context(tc.tile_pool(name="xpool", bufs=3))
    hpool = ctx.enter_context(tc.tile_pool(name="hpool", bufs=2))
    hcpool = ctx.enter_context(tc.tile_pool(name="hcpool", bufs=2))
    upool = ctx.enter_context(tc.tile_pool(name="upool", bufs=2))
    stats = ctx.enter_context(tc.tile_pool(name="stats", bufs=2))
    cols = ctx.enter_context(tc.tile_pool(name="cols", bufs=3))
    dpool = ctx.enter_context(tc.tile_pool(name="dram", bufs=3, space="DRAM"))
    ps_sh = ctx.enter_context(tc.tile_pool(name="ps_sh", bufs=2, space="PSUM"))
    ps_st = ctx.enter_context(tc.tile_pool(name="ps_st", bufs=1, space="PSUM"))
    ps_h1 = ctx.enter_context(tc.tile_pool(name="ps_h1", bufs=1, space="PSUM"))
    ps_h2 = ctx.enter_context(tc.tile_pool(name="ps_h2", bufs=1, space="PSUM"))

    # ---- constants ----
    ones_bf = const.tile([P, 1], BF16)
    nc.vector.memset(ones_bf, 1.0)
    eps_t = const.tile([P, 1], F32)
    nc.vector.memset(eps_t, LN_EPS)
    b1_t = const.tile([P, 4], F32)
    nc.sync.dma_start(out=b1_t, in_=io["b1"].rearrange("(ec p) -> p ec", p=P))
    w1_t = const.tile([P, 4, 4, P], BF16)
    nc.gpsimd.dma_start(
        out=w1_t, in_=io["w1"].rearrange("(dc p) (ec q) -> p dc ec q", p=P, q=P))
    ceacc = const.tile([P, 2 * NF], F32)

    for f in range(NF):
        if f < FC:
            kind, j = "c", f
            w1w, w1r_src = io["wc1"][j], io["wc1r"][j:j + 1]
            w2_src, w2shape = io["wc2"][j], [P, 2, V]
            b2_src, b2n = io["bc2"][j], V
            mask_src = io["maskc"][j]
        elif f < FC + FN:
            kind, j = "n", f - FC
            w1w, w1r_src = io["wn1"][j], io["wn1r"][j:j + 1]
            w2_src, w2shape = io["wn2"][j], [P, 2, D]
            b2_src, b2n = io["bn2"][j], D
            mask_src = io["maskn"][j]
        else:
            kind, j = "b", f - FC - FN
            w1w, w1r_src = io["wb1"][j], io["wb1r"][j:j + 1]
            w2_src, w2shape = io["wb2"][j], [P, 2, 2]
            b2_src, b2n = io["bb2"][j], 2
            mask_src = io["maskb"][j]

        # ---- per-feature constants ----
        w1_f = wpool.tile([P, 4, 2, P], BF16, tag="w1f")
        nc.gpsimd.dma_start(
            out=w1_f, in_=w1w.rearrange("(dc p) (hc q) -> p dc hc q", p=P, q=P))
        w1r = wpool.tile([1, 2, H], BF16, tag="w1r")
        nc.gpsimd.dma_start(out=w1r, in_=w1r_src)
        w2_f = wpool.tile(w2shape, BF16, tag="w2f")
        nc.gpsimd.dma_start(
            out=w2_f, in_=w2_src.rearrange("(hc p) v -> p hc v", p=P))
        b2row = wpool.tile([1, b2n], BF16, tag="b2row")
        nc.gpsimd.dma_start(out=b2row,
                            in_=b2_src.rearrange("(o v) -> o v", o=1))
        mask_t = percf.tile([P, NS], F32, tag="mask")
        nc.sync.dma_start(out=mask_t, in_=mask_src.rearrange("s p -> p s"))
        if kind == "c":
            aux_t = percf.tile([P, NS], F32, tag="aux")
            nc.sync.dma_start(out=aux_t,
                              in_=io["bc2t"][j].rearrange("s p -> p s"))
        elif kind == "b":
            aux_t = percf.tile([P, NS], F32, tag="aux")
            nc.sync.dma_start(out=aux_t,
                              in_=io["boolt"][j].rearrange("s p -> p s"))
        ceb = percf.tile([P, NS], F32, tag="ceb")

        for bt in range(NBT):
            bsl = slice(bt * BT, (bt + 1) * BT)
            # ---- load x^T (cast to bf16 in DMA) ----
            xt_t = xpool.tile([P, 4, BT], BF16)
            nc.gpsimd.dma_start(
                out=xt_t,
                in_=io["xt"][f][:, bsl].rearrange("(dc p) b -> p dc b", p=P))

            # ---- shared layer: matmul + GELU(+b1) ----
            hraw = hpool.tile([P, 4, BT], BF16, tag="hraw")
            for ec in range(4):
                psh = ps_sh.tile([P, BT], F32, tag="psh")
                for dc in range(4):
                    nc.tensor.matmul(psh, w1_t[:, dc, ec, :], xt_t[:, dc, :],
                                     start=(dc == 0), stop=(dc == 3))
                nc.scalar.activation(hraw[:, ec, :], psh, AF.Gelu,
                                     bias=b1_t[:, ec:ec + 1])

            # ---- LN stats over partitions via PE ----
            h2 = hpool.tile([P, 4, BT], BF16, tag="h2")
            for ec in range(4):
                nc.vector.tensor_mul(h2[:, ec, :], hraw[:, ec, :],
                                     hraw[:, ec, :])
            pst = ps_st.tile([1, 2 * BT], F32, tag="strow")
            for ec in range(4):
                nc.tensor.matmul(pst[:, 0:BT], ones_bf, hraw[:, ec, :],
                                 start=(ec == 0), stop=(ec == 3))
            for ec in range(4):
                nc.tensor.matmul(pst[:, BT:2 * BT], ones_bf, h2[:, ec, :],
                                 start=(ec == 0), stop=(ec == 3))
            rows = stats.tile([1, 2 * BT], F32, tag="rows")
            nc.scalar.activation(rows[0:1, 0:BT], pst[0:1, 0:BT], AF.Copy,
                                 scale=1.0 / D)
            nc.scalar.activation(rows[0:1, BT:2 * BT], pst[0:1, BT:2 * BT],
                                 AF.Copy, scale=1.0 / D)
            murow = stats.tile([1, BT], BF16, tag="murow")
            nc.scalar.activation(murow, pst[0:1, 0:BT], AF.Copy, scale=1.0 / D)

            dstat = dpool.tile([1, 2 * BT], F32, tag="dstat")
            nc.sync.dma_start(out=dstat, in_=rows)
            colst = stats.tile([P, 2, NBS], F32, tag="cols")
            nc.sync.dma_start(
                out=colst,
                in_=dstat.rearrange("o (q bs p) -> (o p) q bs",
                                    q=2, bs=NBS, p=P))
            mu_col = colst[:, 0, :]
            ex2_col = colst[:, 1, :]
            tmp = stats.tile([P, NBS], F32, tag="tmp")
            nc.vector.tensor_mul(tmp, mu_col, mu_col)
            var = stats.tile([P, NBS], F32, tag="var")
            nc.vector.tensor_sub(var, ex2_col, tmp)
            secol = stats.tile([P, NBS], F32, tag="secol")
            nc.scalar.activation(secol, var, AF.Sqrt, bias=eps_t[:, 0:1])
            lamcol = stats.tile([P, NBS], F32, tag="lam")
            nc.vector.reciprocal(lamcol, secol)
            dse = dpool.tile([1, BT], F32, tag="dse")
            nc.sync.dma_start(
                out=dse.rearrange("o (bs p) -> (o p) bs", bs=NBS, p=P),
                in_=secol)
            serow = stats.tile([1, BT], BF16, tag="serow")
            nc.gpsimd.dma_start(out=serow, in_=dse)

            # ---- head layer 1 (rank-1 rows carry -mu and se=1/lambda) ----
            psh1 = ps_h1.tile([P, 2, BT], F32, tag="psh1")
            for hc in range(2):
                for dc in range(4):
                    nc.tensor.matmul(psh1[:, hc, :], w1_f[:, dc, hc, :],
                                     hraw[:, dc, :], start=(dc == 0),
                                     stop=False)
                nc.tensor.matmul(psh1[:, hc, :],
                                 w1r[0:1, 0, hc * P:(hc + 1) * P],
                                 murow, start=False, stop=False)
                nc.tensor.matmul(psh1[:, hc, :],
                                 w1r[0:1, 1, hc * P:(hc + 1) * P],
                                 serow, start=False, stop=True)
            hcT = hcpool.tile([P, 2, BT], BF16, tag="hcT")
            for hc in range(2):
                nc.scalar.activation(hcT[:, hc, :], psh1[:, hc, :], AF.Relu)

            if kind == "c":
                # picked logit numerator: sum_h hcT * wct
                wct_t = xpool.tile([P, 2, BT], BF16, tag="wct")
                nc.gpsimd.dma_start(
                    out=wct_t,
                    in_=io["wct"][j][:, bsl].rearrange("(hc p) b -> p hc b",
                                                       p=P))
                prod = hcpool.tile([P, 2, BT], BF16, tag="prod")
                for hc in range(2):
                    nc.vector.tensor_mul(prod[:, hc, :], hcT[:, hc, :],
                                         wct_t[:, hc, :])
                pspk = ps_st.tile([1, BT], F32, tag="strow")
                nc.tensor.matmul(pspk, ones_bf, prod[:, 0, :], start=True,
                                 stop=False)
                nc.tensor.matmul(pspk, ones_bf, prod[:, 1, :], start=False,
                                 stop=True)
                qtrow = stats.tile([1, BT], F32, tag="qtrow")
                nc.scalar.activation(qtrow, pspk, AF.Copy)
                dqt = dpool.tile([1, BT], F32, tag="dqt")
                nc.sync.dma_start(out=dqt, in_=qtrow)
                qtcol = stats.tile([P, NBS], F32, tag="qtcol")
                nc.sync.dma_start(
                    out=qtcol,
                    in_=dqt.rearrange("o (bs p) -> (o p) bs", bs=NBS, p=P))

            # ---- head layer 2 + loss, per 128-row subtile ----
            for bs in range(NBS):
                sidx = bt * NBS + bs
                bpart = slice(bs * P, (bs + 1) * P)
                lam = lamcol[:, bs:bs + 1]
                serow_l = serow[0:1, bpart]
                mk = mask_t[:, sidx:sidx + 1]

                if kind == "c":
                    psq = ps_h2.tile([P, V], F32, tag="psq")
                    for vsl in (slice(0, 512), slice(512, V)):
                        nc.tensor.matmul(psq[:, vsl], hcT[:, 0, bpart],
                                         w2_f[:, 0, vsl], start=True,
                                         stop=False)
                        nc.tensor.matmul(psq[:, vsl], hcT[:, 1, bpart],
                                         w2_f[:, 1, vsl], start=False,
                                         stop=False)
                        nc.tensor.matmul(psq[:, vsl], serow_l,
                                         b2row[0:1, vsl],
                                         start=False, stop=True)
                    u = upool.tile([P, V], F32, tag="u")
                    scol = cols.tile([P, 2], F32, tag="scol")
                    nc.scalar.activation(u[:, 0:512], psq[:, 0:512], AF.Exp,
                                         scale=lam, accum_out=scol[:, 0:1])
                    nc.scalar.activation(u[:, 512:V], psq[:, 512:V], AF.Exp,
                                         scale=lam, accum_out=scol[:, 1:2])
                    ssum = cols.tile([P, 1], F32, tag="ssum")
                    nc.vector.tensor_add(ssum, scol[:, 0:1], scol[:, 1:2])
                    lns = cols.tile([P, 1], F32, tag="lns")
                    nc.scalar.activation(lns, ssum, AF.Ln)
                    t1 = cols.tile([P, 1], F32, tag="t1")
                    nc.vector.scalar_tensor_tensor(
                        out=t1, in0=qtcol[:, bs:bs + 1], scalar=lam,
                        in1=aux_t[:, sidx:sidx + 1],
                        op0=OP.mult, op1=OP.add)
                    nc.vector.scalar_tensor_tensor(
                        out=ceb[:, sidx:sidx + 1], in0=lns, scalar=t1,
                        in1=mk, op0=OP.subtract, op1=OP.mult)
                elif kind == "n":
                    psq = ps_h2.tile([P, D], F32, tag="psq")
                    nc.tensor.matmul(psq, hcT[:, 0, bpart], w2_f[:, 0, :],
                                     start=True, stop=False)
                    nc.tensor.matmul(psq, hcT[:, 1, bpart], w2_f[:, 1, :],
                                     start=False, stop=False)
                    nc.tensor.matmul(psq, serow_l, b2row,
                                     start=False, stop=True)
                    tg = upool.tile([P, D], F32, tag="tg")
                    nc.sync.dma_start(
                        out=tg,
                        in_=io["tgt"][j][bt * BT + bs * P:
                                         bt * BT + (bs + 1) * P, :])
                    diff = upool.tile([P, D], F32, tag="diff")
                    nc.vector.scalar_tensor_tensor(
                        out=diff, in0=psq, scalar=lam, in1=tg,
                        op0=OP.mult, op1=OP.subtract)
                    sq = upool.tile([P, D], BF16, tag="sq")
                    sse = cols.tile([P, 1], F32, tag="sse")
                    nc.vector.scalar_tensor_tensor(
                        out=sq, in0=diff, scalar=1.0, in1=diff,
                        op0=OP.bypass, op1=OP.mult, accum_out=sse)
                    nc.vector.scalar_tensor_tensor(
                        out=ceb[:, sidx:sidx + 1], in0=sse, scalar=1.0 / D,
                        in1=mk, op0=OP.mult, op1=OP.mult)
                else:
                    psq = ps_h2.tile([P, 2], F32, tag="psq")
                    nc.tensor.matmul(psq, hcT[:, 0, bpart], w2_f[:, 0, :],
                                     start=True, stop=False)
                    nc.tensor.matmul(psq, hcT[:, 1, bpart], w2_f[:, 1, :],
                                     start=False, stop=False)
                    nc.tensor.matmul(psq, serow_l, b2row,
                                     start=False, stop=True)
                    u2 = cols.tile([P, 2], F32, tag="u2")
                    s2 = cols.tile([P, 1], F32, tag="scol2")
                    nc.scalar.activation(u2, psq, AF.Exp, scale=lam,
                                         accum_out=s2)
                    lns = cols.tile([P, 1], F32, tag="lns")
                    nc.scalar.activation(lns, s2, AF.Ln)
                    lsb = cols.tile([P, 2], F32, tag="lsb")
                    nc.vector.tensor_scalar_mul(lsb, psq, 1.0)
                    dlt = cols.tile([P, 1], F32, tag="dlt")
                    nc.vector.tensor_sub(dlt, lsb[:, 1:2], lsb[:, 0:1])
                    p2t = cols.tile([P, 1], F32, tag="p2t")
                    nc.vector.scalar_tensor_tensor(
                        out=p2t, in0=dlt, scalar=aux_t[:, sidx:sidx + 1],
                        in1=lsb[:, 0:1], op0=OP.mult, op1=OP.add)
                    t1 = cols.tile([P, 1], F32, tag="t1")
                    nc.vector.tensor_scalar_mul(t1, p2t, lam)
                    nc.vector.scalar_tensor_tensor(
                        out=ceb[:, sidx:sidx + 1], in0=lns, scalar=t1,
                        in1=mk, op0=OP.subtract, op1=OP.mult)

        # ---- per-feature reduction over subtiles ----
        nc.vector.reduce_sum(ceacc[:, f:f + 1], ceb, axis=mybir.AxisListType.X)
        nc.vector.reduce_sum(ceacc[:, NF + f:NF + f + 1], mask_t,
                             axis=mybir.AxisListType.X)

    # ---- final partition reduction via DRAM-transpose + DVE ----
    dfin = dpool.tile([P, 2 * NF], F32, tag="dfin")
    nc.sync.dma_start(out=dfin, in_=ceacc)
    fin = stats.tile([2 * NF, P], F32, tag="fin")
    nc.sync.dma_start(out=fin, in_=dfin.rearrange("p c -> c p"))
    outc = stats.tile([2 * NF, 1], F32, tag="outc")
    nc.vector.reduce_sum(outc, fin, axis=mybir.AxisListType.X)
    nc.sync.dma_start(out=out.rearrange("(p o) -> p o", o=1), in_=outc)


_NC_CACHE = None


def _get_nc():
    global _NC_CACHE
    if _NC_CACHE is None:
        _NC_CACHE = _build()
    return _NC_CACHE


def _prep_core(i, seq, targets, mask_f, cat_t, bool_t, w):
    """Build the in_map for core i. All layout / slicing, no data math."""
    cg = list(range(4 * i, 4 * i + 4))
    ng = list(range(2 * i, 2 * i + 2))
    bg = list(range(2 * i, 2 * i + 2))
    feats = cg + [NC + g for g in ng] + [NC + NN + g for g in bg]

    xt = np.ascontiguousarray(
        seq[:, feats, :].transpose(1, 2, 0)).astype(np.float32)

    m = {
        "xt": xt,
        "w1": w["W1"],
        "b1": w["b1"],
        "wc1": w["wc1p"][cg],
        "wc1r": w["wc1r"][cg],
        "wc2": w["Wc2"][cg],
        "bc2": w["bc2"][cg],
        "wct": np.ascontiguousarray(
            np.stack([w["Wc2"][g][:, cat_t[:, g]] for g in cg])),
        "bc2t": np.stack([w["bc2"][g][cat_t[:, g]].reshape(NS, P)
                          for g in cg]),
        "maskc": np.stack([mask_f[:, g].reshape(NS, P) for g in cg]),
        "wn1": w["wn1p"][ng],
        "wn1r": w["wn1r"][ng],
        "wn2": w["Wn2"][ng],
        "bn2": w["bn2"][ng],
        "tgt": np.ascontiguousarray(
            targets[:, [NC + g for g in ng], :].transpose(1, 0, 2)),
        "maskn": np.stack([mask_f[:, NC + g].reshape(NS, P) for g in ng]),
        "wb1": w["wb1p"][bg],
        "wb1r": w["wb1r"][bg],
        "wb2": w["Wb2"][bg],
        "bb2": w["bb2"][bg],
        "boolt": np.stack([bool_t[:, g].astype(np.float32).reshape(NS, P)
                           for g in bg]),
        "maskb": np.stack([mask_f[:, NC + NN + g].reshape(NS, P)
                           for g in bg]),
    }
    return {k: np.ascontiguousarray(v, dtype=np.float32) for k, v in m.items()}


def prepare_in_maps(inputs):
    seq = np.asarray(inputs["sequence_embeddings"], np.float32)
    targets = np.asarray(inputs["targets"], np.float32)
    mask_f = np.asarray(inputs["target_mask"]).astype(np.float32)
    cat_t = np.asarray(inputs["cat_targets"]).astype(np.int64)
    bool_t = np.asarray(inputs["bool_targets"]).astype(np.int64)

    ln_g = np.asarray(inputs["ln_g"], np.float64)
    ln_b = np.asarray(inputs["ln_b"], np.float64)

    def fold(w1, b1):
        w1 = np.asarray(w1, np.float64)
        b1 = np.asarray(b1, np.float64)
        wp = ln_g[None, :, None] * w1                    # [F, D, H]
        bp = b1 + np.einsum("d,fdh->fh", ln_b, w1)       # [F, H]
        rows = np.stack([-wp.sum(1), bp], axis=1)        # [F, 2, H]
        return wp.astype(np.float32), rows.astype(np.float32)

    w = {
        "W1": np.asarray(inputs["W1"], np.float32),
        "b1": np.asarray(inputs["b1"], np.float32),
        "Wc2": np.asarray(inputs["Wc2"], np.float32),
        "bc2": np.asarray(inputs["bc2"], np.float32),
        "Wn2": np.asarray(inputs["Wn2"], np.float32),
        "bn2": np.asarray(inputs["bn2"], np.float32),
        "Wb2": np.asarray(inputs["Wb2"], np.float32),
        "bb2": np.asarray(inputs["bb2"], np.float32),
    }
    w["wc1p"], w["wc1r"] = fold(inputs["Wc1"], inputs["bc1"])
    w["wn1p"], w["wn1r"] = fold(inputs["Wn1"], inputs["bn1"])
    w["wb1p"], w["wb1r"] = fold(inputs["Wb1"], inputs["bb1"])

    return [_prep_core(i, seq, targets, mask_f, cat_t, bool_t, w)
            for i in range(N_CORES)]


def combine(per_core_outs):
    total = 0.0
    for r in per_core_outs:
        r = np.asarray(r, np.float64)
        s, c = r[:NF], r[NF:]
        total += np.where(c > 0, s / np.maximum(c, 1.0), 0.0).sum()
    return np.float32(total)


def kernel(**inputs):
    global LAST_RESULTS
    in_maps = prepare_in_maps(inputs)
    nc = _get_nc()
    res = run_bass_kernel_spmd(nc, in_maps, core_ids=list(range(N_CORES)))
    LAST_RESULTS = res
    return combine([res.results[i]["loss_out"] for i in range(N_CORES)])



# revision 2
# speedup vs baseline: 159.2238x; 159.2238x over previous
"""Trainium2 Bass kernel for nn_NextRowPredictionHead (loss_fn) — V3 (fp8).

Feature-parallel across 8 cores (4 cat + 2 num + 2 bool features each,
full batch). V2 restructures the device schedule into per-batch-tile
phases so the Activation engine loads each activation-function table
once per phase instead of cycling Gelu/Sqrt/Exp/Ln per feature
(~1.3us per table reload), ships all large inputs as bf16, and keeps
PSUM-row copies off the ACT engine.

Per batch tile of 512 samples:
  phase A (gelu table): per feature: shared matmul + GELU, h^2, LN
    stats via ones-matmuls; stats rows copied out via DVE and
    redistributed to columns through a DRAM bounce.
  phase S (sqrt table): one batched Sqrt over all features' variances,
    DVE reciprocal, std rows redistributed via DRAM bounce.
  phase B1 (exp table): per feature: head1 matmuls with rank-1 mu/std
    rows, ReLU, head2 matmuls with rank-1 std*b2 row, Exp+accumulate
    (softmax denominators), picked-logit path for categorical, MSE for
    numerical (DVE only).
  phase B2 (ln): batched Ln ops + DVE cross-entropy assembly.
Losses accumulate per feature as masked per-sample columns; the final
reduction is a f32 ones-matmul over partitions.
"""

import sys
from contextlib import ExitStack

import numpy as np
import ml_dtypes

sys.path.insert(0, "/opt/trn_rl_repo")

import concourse.bass as bass  # noqa: E402,F401
import concourse.tile as tile  # noqa: E402
from concourse import bacc, mybir  # noqa: E402
from concourse.bass_utils import run_bass_kernel_spmd  # noqa: E402

F32 = mybir.dt.float32
BF16 = mybir.dt.bfloat16
AF = mybir.ActivationFunctionType
OP = mybir.AluOpType
AX = mybir.AxisListType
BF = ml_dtypes.bfloat16
F8 = mybir.dt.float8e4
F8H = ml_dtypes.float8_e4m3
WS = 16.0      # weight pre-scale shipped from host (W1-side and W2-side)
VP = 1024      # padded categorical vocab (DoubleRow stride alignment)

P = 128
D = 512
H = 256
V = 1000
B = 2048
NC, NN, NB = 32, 16, 16
FC, FN, FB = 4, 2, 2
NF = FC + FN + FB
NBT = 4
BT = 512
NBS = 4
NS = NBT * NBS
LN_EPS = 1e-5
N_CORES = 8

LAST_RESULTS = None


def _build():
    nc = bacc.Bacc("TRN2", target_bir_lowering=False, debug=False,
                   num_devices=N_CORES)
    io = {}

    def din(name, shape, dt=BF16):
        io[name] = nc.dram_tensor(name, shape, dt, kind="ExternalInput").ap()

    din("xt", [NF, P, 4, B], F8)
    din("w1", [P, 4 * 4 * P], F8)
    din("b1", [P, 4], F32)
    din("w1p", [P, NF * 4 * 2 * P], F8)
    din("w1r", [1, NF * 2 * H])
    din("w2c", [P, FC * 2 * VP], F8)
    din("w2n", [P, FN * 2 * D], F8)
    din("w2b", [P, FB * 2 * 2], F8)
    din("browc", [1, FC, VP])
    din("brown", [1, FN, D])
    din("browb", [1, FB, 2])
    din("wct", [FC, P, 2, B], F8)
    din("tgt", [FN, B, D])
    din("maskall", [P, NF, NS], F32)
    din("auxall", [P, NF, NS], F32)
    out = nc.dram_tensor("loss_out", [2 * NF], F32, kind="ExternalOutput").ap()

    with tile.TileContext(nc) as tc:
        with ExitStack() as ctx:
            build_body(ctx, tc, io, out)
    nc.compile()
    return nc


def build_body(ctx, tc, io, out):
    nc = tc.nc

    const = ctx.enter_context(tc.tile_pool(name="const", bufs=1))
    pers = ctx.enter_context(tc.tile_pool(name="pers", bufs=1))
    abuf = ctx.enter_context(tc.tile_pool(name="abuf", bufs=2))
    sbuf2 = ctx.enter_context(tc.tile_pool(name="sbuf2", bufs=2))
    dpool = ctx.enter_context(tc.tile_pool(name="dram", bufs=2, space="DRAM"))
    ps_sh = ctx.enter_context(tc.tile_pool(name="ps_sh", bufs=2, space="PSUM"))
    ps_st = ctx.enter_context(tc.tile_pool(name="ps_st", bufs=1, space="PSUM"))
    ps_h1 = ctx.enter_context(tc.tile_pool(name="ps_h1", bufs=1, space="PSUM"))
    ps_q = ctx.enter_context(tc.tile_pool(name="ps_q", bufs=2, space="PSUM"))

    # ---- constants ----
    ones_st = const.tile([P, 2, 16], F8)
    nc.vector.memset(ones_st, 1.0 / D)
    ones_pk = const.tile([P, 1], BF16)
    nc.vector.memset(ones_pk, 1.0)
    ones_f = const.tile([P, 1], F32)
    nc.vector.memset(ones_f, 1.0)
    eps_t = const.tile([P, 1], F32)
    nc.vector.memset(eps_t, LN_EPS)
    b1_t = const.tile([P, 4], F32)
    nc.sync.dma_start(out=b1_t, in_=io["b1"])
    w1_f = const.tile([P, 4 * 4 * P], F8, name="w1f")
    nc.sync.dma_start(out=w1_f, in_=io["w1"])
    w1_t = w1_f.rearrange("p (a e q) -> p a e q", a=4, e=4, q=P)
    w1p_f = const.tile([P, NF * 4 * 2 * P], F8, name="w1pf")
    w1p = w1p_f.rearrange("p (f a h q) -> p f a h q", f=NF, a=4, h=2, q=P)
    w1r_f = const.tile([1, NF * 2 * H], BF16, name="w1rf")
    w1r = w1r_f.rearrange("o (f a h) -> o f a h", f=NF, a=2, h=H)
    w2c_f = const.tile([P, FC * 2 * VP], F8, name="w2cf")
    w2c = w2c_f.rearrange("p (j h v) -> p j h v", j=FC, h=2, v=VP)
    w2n_f = const.tile([P, FN * 2 * D], F8, name="w2nf")
    w2n = w2n_f.rearrange("p (j h d) -> p j h d", j=FN, h=2, d=D)
    w2b_f = const.tile([P, FB * 2 * 2], F8, name="w2bf")
    w2b = w2b_f.rearrange("p (j h d) -> p j h d", j=FB, h=2, d=2)
    browc = const.tile([1, FC, VP], BF16)
    brown = const.tile([1, FN, D], BF16)
    browb = const.tile([1, FB, 2], BF16)
    maskall = const.tile([P, NF, NS], F32)
    auxall = const.tile([P, NF, NS], F32)
    cebs = [const.tile([P, NS], F32, tag=f"ceb{f}", name=f"ceb{f}")
            for f in range(NF)]

    def emit_A(bt, gate):
        """Phase A+S for batch tile bt: shared layer, LN stats, std rows.
        Returns per-tile state consumed by emit_B."""
        bsl = slice(bt * BT, (bt + 1) * BT)
        st = {"hraws": [], "srows": []}
        var_all = sbuf2.tile([P, NF, NBS], F32, tag="var", name="var_all")
        for f in range(NF):
            xt_t = abuf.tile([P, 4, BT], F8, tag="xt", name="xt_t")
            nc.sync.dma_start(out=xt_t, in_=io["xt"][f][:, :, bsl])
            hraw = pers.tile([P, 4, BT], F8, tag=f"hraw{f}", bufs=2,
                             name="hraw")
            for ec in range(4):
                psh = ps_sh.tile([P, BT], F32, tag="psh", name="psh")
                for dp in range(2):
                    nc.tensor.matmul(
                        psh, w1_t[:, 2 * dp:2 * dp + 2, ec, :],
                        xt_t[:, 2 * dp:2 * dp + 2, :],
                        start=(dp == 0), stop=(dp == 1),
                        perf_mode=mybir.MatmulPerfMode.DoubleRow)
                nc.scalar.activation(hraw[:, ec, :], psh, AF.Gelu,
                                     bias=gate[:, ec:ec + 1],
                                     scale=1.0 / WS)
            h2 = abuf.tile([P, 4, BT], F8, tag="h2", name="h2")
            nc.gpsimd.tensor_mul(h2, hraw, hraw)
            pst = ps_st.tile([1, 2 * BT], F32, tag="pst", name="pst")
            for dp in range(2):
                nc.tensor.matmul(pst[:, 0:BT], ones_st[:, :, 0:1],
                                 hraw[:, 2 * dp:2 * dp + 2, :],
                                 start=(dp == 0), stop=(dp == 1),
                                 perf_mode=mybir.MatmulPerfMode.DoubleRow)
            for dp in range(2):
                nc.tensor.matmul(pst[:, BT:2 * BT], ones_st[:, :, 0:1],
                                 h2[:, 2 * dp:2 * dp + 2, :],
                                 start=(dp == 0), stop=(dp == 1),
                                 perf_mode=mybir.MatmulPerfMode.DoubleRow)
            sr = pers.tile([1, 2 * BT], BF16, tag=f"srow{f}", bufs=2,
                           name="sr")
            nc.vector.tensor_scalar_mul(sr, pst, 1.0)
            dstat = dpool.tile([1, 2 * BT], BF16, tag="dstat", name="dstat")
            nc.sync.dma_start(out=dstat, in_=sr)
            colst = sbuf2.tile([P, 2, NBS], BF16, tag="colst", name="colst")
            nc.sync.dma_start(
                out=colst,
                in_=dstat.rearrange("o (q bs p) -> (o p) q bs",
                                    q=2, bs=NBS, p=P))
            musq = sbuf2.tile([P, NBS], F32, tag="musq", name="musq")
            nc.gpsimd.tensor_mul(musq, colst[:, 0, :], colst[:, 0, :])
            nc.gpsimd.tensor_sub(var_all[:, f, :], colst[:, 1, :], musq)
            st["hraws"].append(hraw)
            st["srows"].append(sr)

        # phase S: std + 1/std
        secol = sbuf2.tile([P, NF, NBS], BF16, tag="secol", name="secol")
        nc.scalar.activation(secol, var_all, AF.Sqrt, bias=eps_t[:, 0:1])
        lam = sbuf2.tile([P, NF, NBS], F32, tag="lam", name="lam")
        nc.vector.reciprocal(lam, secol)
        lam256 = sbuf2.tile([P, NF, NBS], F32, tag="lam256", name="lam256")
        nc.gpsimd.tensor_scalar_mul(lam256, lam, 1.0 / (WS * WS))
        dse = dpool.tile([1, NF, BT], BF16, tag="dse", name="dse")
        nc.sync.dma_start(
            out=dse.rearrange("o f (bs p) -> (o p) f bs", bs=NBS, p=P),
            in_=secol)
        serow = sbuf2.tile([1, NF, BT], BF16, tag="serow", name="serow")
        nc.sync.dma_start(out=serow, in_=dse)
        st["lam256"] = lam256
        st["serow"] = serow
        return st

    def emit_B(bt, st, make_gate):
        """Phase B for batch tile bt: heads, exp, ln, CE assembly."""
        bsl = slice(bt * BT, (bt + 1) * BT)
        csl = slice(bt * NBS, (bt + 1) * NBS)
        lam256 = st["lam256"]
        serow = st["serow"]
        lnall = sbuf2.tile([P, FC + FB, NBS], F32, tag="lnall", name="lnall")
        t1cs = {}
        t1bs = {}
        for f in range(NF):
            hraw = st["hraws"][f]
            murow = st["srows"][f][0:1, 0:BT]
            serow_f = serow[0:1, f, :]
            psh1 = ps_h1.tile([P, 2, BT], F32, tag="psh1", name="psh1")
            for hc in range(2):
                for dp in range(2):
                    nc.tensor.matmul(
                        psh1[:, hc, :],
                        w1p[:, f, 2 * dp:2 * dp + 2, hc, :],
                        hraw[:, 2 * dp:2 * dp + 2, :],
                        start=(dp == 0), stop=False,
                        perf_mode=mybir.MatmulPerfMode.DoubleRow)
                nc.tensor.matmul(psh1[:, hc, :],
                                 w1r[0:1, f, 0, hc * P:(hc + 1) * P],
                                 murow, start=False, stop=False)
                nc.tensor.matmul(psh1[:, hc, :],
                                 w1r[0:1, f, 1, hc * P:(hc + 1) * P],
                                 serow_f, start=False, stop=True)
            hcT = abuf.tile([P, 2, BT], F8, tag="hcT", name="hcT")
            for hc in range(2):
                nc.vector.tensor_scalar_max(hcT[:, hc, :], psh1[:, hc, :],
                                            0.0)

            if f < FC:
                j = f
                wct_t = abuf.tile([P, 2, BT], F8, tag="wct", name="wct_t")
                nc.sync.dma_start(out=wct_t, in_=io["wct"][j][:, :, bsl])
                prod = abuf.tile([P, 2, BT], BF16, tag="prod", name="prod")
                nc.gpsimd.tensor_mul(prod, hcT, wct_t)
                psqt = ps_sh.tile([P, BT], F32, tag="psh", name="psqt")
                nc.tensor.matmul(psqt[0:1, :], ones_pk, prod[:, 0, :],
                                 start=True, stop=False)
                nc.tensor.matmul(psqt[0:1, :], ones_pk, prod[:, 1, :],
                                 start=False, stop=True)
                qtrow = sbuf2.tile([1, BT], F32, tag="qtrow", name="qtrow")
                nc.vector.tensor_scalar_mul(qtrow, psqt[0:1, :], 1.0)
                dqt = dpool.tile([1, BT], F32, tag="dqt", name="dqt")
                nc.sync.dma_start(out=dqt, in_=qtrow)
                qtcol = sbuf2.tile([P, NBS], F32, tag="qtcol", name="qtcol")
                nc.sync.dma_start(
                    out=qtcol,
                    in_=dqt.rearrange("o (bs p) -> (o p) bs", bs=NBS, p=P))
                ssc = pers.tile([P, NBS, 2], F32, tag=f"ssc{f}", name="ssc")
                for bs in range(NBS):
                    bpart = slice(bs * P, (bs + 1) * P)
                    for vi in range(2):
                        vsl = slice(vi * 512, (vi + 1) * 512)
                        nv = 512 if vi == 0 else V - 512
                        psq = ps_q.tile([P, 512], F32, tag="psq", name="psq")
                        nc.tensor.matmul(
                            psq, hcT[:, :, bpart], w2c[:, j, :, vsl],
                            start=True, stop=False,
                            perf_mode=mybir.MatmulPerfMode.DoubleRow)
                        nc.tensor.matmul(psq, serow_f[0:1, bpart],
                                         browc[0:1, j, vsl],
                                         start=False, stop=True)
                        u = abuf.tile([P, 512], BF16, tag="u", name="u")
                        nc.scalar.activation(u[:, 0:nv], psq[:, 0:nv], AF.Exp,
                                             scale=lam256[:, f, bs:bs + 1],
                                             accum_out=ssc[:, bs, vi:vi + 1])
                t0 = sbuf2.tile([P, NBS], F32, tag="t0", name="t0")
                nc.gpsimd.tensor_mul(t0, qtcol, lam256[:, f, :])
                t1c = pers.tile([P, NBS], F32, tag=f"t1c{f}", name="t1c")
                nc.gpsimd.tensor_add(t1c, t0, auxall[:, f, csl])
                nc.gpsimd.tensor_add(lnall[:, f, :], ssc[:, :, 0],
                                     ssc[:, :, 1])
                t1cs[f] = t1c
            elif f < FC + FN:
                j = f - FC
                tg = abuf.tile([P, NBS, D], BF16, tag="tg", name="tg")
                nc.sync.dma_start(
                    out=tg,
                    in_=io["tgt"][j][bsl].rearrange("(bs p) d -> p bs d",
                                                    bs=NBS, p=P))
                msec = pers.tile([P, NBS], F32, tag=f"msec{f}", name="msec")
                for bs in range(NBS):
                    bpart = slice(bs * P, (bs + 1) * P)
                    psq = ps_q.tile([P, 512], F32, tag="psq", name="psq")
                    nc.tensor.matmul(psq, hcT[:, :, bpart], w2n[:, j, :, :],
                                     start=True, stop=False,
                                     perf_mode=mybir.MatmulPerfMode.DoubleRow)
                    nc.tensor.matmul(psq, serow_f[0:1, bpart],
                                     brown[0:1, j, :],
                                     start=False, stop=True)
                    diff = abuf.tile([P, D], F32, tag="diff", name="diff")
                    nc.vector.scalar_tensor_tensor(
                        out=diff, in0=psq, scalar=lam256[:, f, bs:bs + 1],
                        in1=tg[:, bs, :], op0=OP.mult, op1=OP.subtract)
                    sq = abuf.tile([P, D], BF16, tag="sq", name="sq")
                    nc.vector.scalar_tensor_tensor(
                        out=sq, in0=diff, scalar=1.0, in1=diff,
                        op0=OP.bypass, op1=OP.mult,
                        accum_out=msec[:, bs:bs + 1])
                nc.vector.scalar_tensor_tensor(
                    out=cebs[f][:, csl], in0=msec, scalar=1.0 / D,
                    in1=maskall[:, f, csl], op0=OP.mult, op1=OP.mult)
            else:
                j = f - FC - FN
                psq = ps_q.tile([P, 512], F32, tag="psq", name="psq")
                pb = psq[:, 0:2 * NBS].rearrange("p (bs two) -> p bs two",
                                                 two=2)
                for bs in range(NBS):
                    bpart = slice(bs * P, (bs + 1) * P)
                    nc.tensor.matmul(pb[:, bs, :], hcT[:, :, bpart],
                                     w2b[:, j, :, :], start=True, stop=False,
                                     perf_mode=mybir.MatmulPerfMode.DoubleRow)
                    nc.tensor.matmul(pb[:, bs, :], serow_f[0:1, bpart],
                                     browb[0:1, j, :], start=False, stop=True)
                zb = pers.tile([P, NBS, 2], F32, tag=f"zb{f}", name="zb")
                for bs in range(NBS):
                    nc.vector.tensor_scalar_mul(
                        zb[:, bs, :], pb[:, bs, :], lam256[:, f, bs:bs + 1])
                u2 = sbuf2.tile([P, NBS, 2], F32, tag="u2", name="u2")
                nc.scalar.activation(u2, zb, AF.Exp)
                nc.gpsimd.tensor_add(lnall[:, FC + j, :],
                                     u2[:, :, 0], u2[:, :, 1])
                dlt = sbuf2.tile([P, NBS], F32, tag="dlt", name="dlt")
                nc.gpsimd.tensor_sub(dlt, zb[:, :, 1], zb[:, :, 0])
                ta = sbuf2.tile([P, NBS], F32, tag="ta", name="ta")
                nc.gpsimd.tensor_mul(ta, dlt, auxall[:, f, csl])
                t1b = pers.tile([P, NBS], F32, tag=f"t1b{f}", name="t1b")
                nc.gpsimd.tensor_add(t1b, ta, zb[:, :, 0])
                t1bs[f] = t1b

        # phase B2: one Ln + CE assembly
        lns_all = sbuf2.tile([P, FC + FB, NBS], F32, tag="lns", name="lns")
        nc.scalar.activation(lns_all, lnall, AF.Ln)
        for f in range(NF):
            if FC <= f < FC + FN:
                continue
            k = f if f < FC else f - FN
            t1 = t1cs[f] if f < FC else t1bs[f]
            tmp = sbuf2.tile([P, NBS], F32, tag="tmp", name="tmp")
            nc.gpsimd.tensor_sub(tmp, lns_all[:, k, :], t1)
            nc.gpsimd.tensor_mul(cebs[f][:, csl], tmp, maskall[:, f, csl])
        if not make_gate:
            return None
        # ACT-order gate: tile bt+2's Gelus read the gate, which
        # (bypass-)depends on this tile's Ln output, so the scheduler keeps
        # at most one Gelu phase ahead of the Exp/Ln stream (table thrash).
        b1g = sbuf2.tile([P, 4], F32, tag="b1gate", name="b1g")
        nc.vector.scalar_tensor_tensor(
            out=b1g, in0=b1_t, scalar=1.0, in1=lns_all[:, 0, :],
            op0=OP.mult, op1=OP.bypass)
        return b1g

    # 1-deep software pipeline: A(k+1) is emitted before B(k) so the LN
    # stats round trip of tile k+1 hides under tile k's head phase.
    states = {0: emit_A(0, b1_t)}
    # big weight/const loads queued after tile 0's xt loads so the first
    # Gelus aren't starved behind them on the DMA engines
    nc.sync.dma_start(out=w1p_f, in_=io["w1p"])
    nc.sync.dma_start(out=w2c_f, in_=io["w2c"])
    nc.sync.dma_start(out=w2n_f, in_=io["w2n"])
    nc.sync.dma_start(out=w2b_f, in_=io["w2b"])
    nc.sync.dma_start(out=browc, in_=io["browc"])
    nc.sync.dma_start(out=brown, in_=io["brown"])
    nc.sync.dma_start(out=browb, in_=io["browb"])
    nc.sync.dma_start(out=maskall, in_=io["maskall"])
    nc.sync.dma_start(out=auxall, in_=io["auxall"])
    nc.sync.dma_start(out=w1r_f, in_=io["w1r"])
    for k in range(NBT):
        if k + 1 < NBT:
            gate = b1_t if k < 1 else gates[k - 1]
            states[k + 1] = emit_A(k + 1, gate)
        g = emit_B(k, states.pop(k), make_gate=(k + 2 < NBT))
        if k == 0:
            gates = {}
        if g is not None:
            gates[k] = g

    # ---------------- final reduction --------------------------------
    acc2 = const.tile([P, 2 * NF], F32)
    for f in range(NF):
        nc.vector.reduce_sum(acc2[:, f:f + 1], cebs[f], axis=AX.X)
        nc.vector.reduce_sum(acc2[:, NF + f:NF + f + 1], maskall[:, f, :],
                             axis=AX.X)
    pso = ps_q.tile([P, 512], F32, tag="psq")
    nc.tensor.matmul(pso[0:1, 0:2 * NF], ones_f, acc2, start=True, stop=True)
    outc = const.tile([1, 2 * NF], F32)
    nc.vector.tensor_scalar_mul(outc, pso[0:1, 0:2 * NF], 1.0)
    nc.sync.dma_start(out=out.rearrange("(o c) -> o c", o=1), in_=outc)


_NC_CACHE = None


def _get_nc():
    global _NC_CACHE
    if _NC_CACHE is None:
        _NC_CACHE = _build()
    return _NC_CACHE


def _prep_core(i, seq, targets, mask_f, cat_t, bool_t, w):
    """Build the in_map for core i. All layout / slicing, no data math."""
    cg = list(range(4 * i, 4 * i + 4))
    ng = [2 * i, 2 * i + 1]
    bg = [2 * i, 2 * i + 1]
    feats = cg + [NC + g for g in ng] + [NC + NN + g for g in bg]

    xf = seq[:, feats, :]                                   # [B, NF, D]
    xt = (xf.transpose(1, 2, 0).reshape(NF, 4, P, B)
          .transpose(0, 2, 1, 3))                           # [NF, P, 4, B]

    w1p_all = np.concatenate([w["wc1p"][cg], w["wn1p"][ng], w["wb1p"][bg]])
    w1p = (w1p_all.reshape(NF, 4, P, 2, P)
           .transpose(2, 0, 1, 3, 4))                       # [P, NF, 4, 2, P]
    w1r = np.concatenate([w["wc1r"][cg], w["wn1r"][ng], w["wb1r"][bg]])

    wc2p = np.zeros((FC, H, VP), np.float32)
    wc2p[:, :, :V] = w["Wc2"][cg]
    w2c = wc2p.reshape(FC, 2, P, VP).transpose(2, 0, 1, 3)
    w2n = w["Wn2"][ng].reshape(FN, 2, P, D).transpose(2, 0, 1, 3)
    w2b = w["Wb2"][bg].reshape(FB, 2, P, 2).transpose(2, 0, 1, 3)

    wct = np.stack([w["Wc2"][g][:, cat_t[:, g]].reshape(2, P, B)
                    .transpose(1, 0, 2) for g in cg])       # [FC, P, 2, B]

    tgt = targets[:, [NC + g for g in ng], :].transpose(1, 0, 2)  # [FN, B, D]

    maskall = (mask_f[:, feats].reshape(NS, P, NF)
               .transpose(1, 2, 0))                         # [P, NF, NS]
    auxall = np.zeros((P, NF, NS), np.float32)
    for k, g in enumerate(cg):
        auxall[:, k, :] = w["bc2"][g][cat_t[:, g]].reshape(NS, P).T
    for k, g in enumerate(bg):
        auxall[:, FC + FN + k, :] = bool_t[:, g].astype(
            np.float32).reshape(NS, P).T

    bc2p = np.zeros((FC, VP), np.float32)
    bc2p[:, :V] = w["bc2"][cg]
    m8 = {
        "xt": xt,
        "w1": w["w1h"].reshape(P, -1) * WS,
        "w1p": w1p.reshape(P, -1) * WS,
        "w2c": w2c.reshape(P, -1) * WS,
        "w2n": w2n.reshape(P, -1) * WS,
        "w2b": w2b.reshape(P, -1) * WS,
        "wct": wct * WS,
    }
    m16 = {
        "w1r": w1r.reshape(1, -1) * WS,
        "browc": bc2p[None] * (WS * WS),
        "brown": w["bn2"][ng][None] * (WS * WS),
        "browb": w["bb2"][bg][None] * (WS * WS),
        "tgt": tgt,
    }
    m = {k: np.ascontiguousarray(v.astype(F8H)) for k, v in m8.items()}
    m.update({k: np.ascontiguousarray(v.astype(BF)) for k, v in m16.items()})
    m["b1"] = np.ascontiguousarray(w["b1h"], np.float32)
    m["maskall"] = np.ascontiguousarray(maskall, np.float32)
    m["auxall"] = np.ascontiguousarray(auxall, np.float32)
    return m


def prepare_in_maps(inputs):
    seq = np.asarray(inputs["sequence_embeddings"], np.float32)
    targets = np.asarray(inputs["targets"], np.float32)
    mask_f = np.asarray(inputs["target_mask"]).astype(np.float32)
    cat_t = np.asarray(inputs["cat_targets"]).astype(np.int64)
    bool_t = np.asarray(inputs["bool_targets"]).astype(np.int64)

    ln_g = np.asarray(inputs["ln_g"], np.float64)
    ln_b = np.asarray(inputs["ln_b"], np.float64)

    def fold(w1, b1):
        w1 = np.asarray(w1, np.float64)
        b1 = np.asarray(b1, np.float64)
        wp = ln_g[None, :, None] * w1                    # [F, D, H]
        bp = b1 + np.einsum("d,fdh->fh", ln_b, w1)       # [F, H]
        rows = np.stack([-wp.sum(1), bp], axis=1)        # [F, 2, H]
        return wp.astype(np.float32), rows.astype(np.float32)

    W1 = np.asarray(inputs["W1"], np.float32)
    b1 = np.asarray(inputs["b1"], np.float32)
    w = {
        "w1h": W1.reshape(4, P, 4, P).transpose(1, 0, 2, 3),
        "b1h": b1.reshape(4, P).T,
        "Wc2": np.asarray(inputs["Wc2"], np.float32),
        "bc2": np.asarray(inputs["bc2"], np.float32),
        "Wn2": np.asarray(inputs["Wn2"], np.float32),
        "bn2": np.asarray(inputs["bn2"], np.float32),
        "Wb2": np.asarray(inputs["Wb2"], np.float32),
        "bb2": np.asarray(inputs["bb2"], np.float32),
    }
    w["wc1p"], w["wc1r"] = fold(inputs["Wc1"], inputs["bc1"])
    w["wn1p"], w["wn1r"] = fold(inputs["Wn1"], inputs["bn1"])
    w["wb1p"], w["wb1r"] = fold(inputs["Wb1"], inputs["bb1"])

    return [_prep_core(i, seq, targets, mask_f, cat_t, bool_t, w)
            for i in range(N_CORES)]


def combine(per_core_outs):
    total = 0.0
    for r in per_core_outs:
        r = np.asarray(r, np.float64)
        s, c = r[:NF], r[NF:]
        total += np.where(c > 0, s / np.maximum(c, 1.0), 0.0).sum()
    return np.float32(total)


def kernel(**inputs):
    global LAST_RESULTS
    in_maps = prepare_in_maps(inputs)
    nc = _get_nc()
    res = run_bass_kernel_spmd(nc, in_maps, core_ids=list(range(N_CORES)))
    LAST_RESULTS = res
    return combine([res.results[i]["loss_out"] for i in range(N_CORES)])


# revision 3
# speedup vs baseline: 168.1381x; 1.0560x over previous
"""Trainium2 Bass kernel for nn_NextRowPredictionHead (loss_fn) — V3 (fp8).

Feature-parallel across 8 cores (4 cat + 2 num + 2 bool features each,
full batch). V2 restructures the device schedule into per-batch-tile
phases so the Activation engine loads each activation-function table
once per phase instead of cycling Gelu/Sqrt/Exp/Ln per feature
(~1.3us per table reload), ships all large inputs as bf16, and keeps
PSUM-row copies off the ACT engine.

Per batch tile of 512 samples:
  phase A (gelu table): per feature: shared matmul + GELU, h^2, LN
    stats via ones-matmuls; stats rows copied out via DVE and
    redistributed to columns through a DRAM bounce.
  phase S (sqrt table): one batched Sqrt over all features' variances,
    DVE reciprocal, std rows redistributed via DRAM bounce.
  phase B1 (exp table): per feature: head1 matmuls with rank-1 mu/std
    rows, ReLU, head2 matmuls with rank-1 std*b2 row, Exp+accumulate
    (softmax denominators), picked-logit path for categorical, MSE for
    numerical (DVE only).
  phase B2 (ln): batched Ln ops + DVE cross-entropy assembly.
Losses accumulate per feature as masked per-sample columns; the final
reduction is a f32 ones-matmul over partitions.
"""

import sys
from contextlib import ExitStack

import numpy as np
import ml_dtypes

sys.path.insert(0, "/opt/trn_rl_repo")

import concourse.bass as bass  # noqa: E402,F401
import concourse.tile as tile  # noqa: E402
from concourse import bacc, mybir  # noqa: E402
from concourse.bass_utils import run_bass_kernel_spmd  # noqa: E402

F32 = mybir.dt.float32
BF16 = mybir.dt.bfloat16
AF = mybir.ActivationFunctionType
OP = mybir.AluOpType
AX = mybir.AxisListType
BF = ml_dtypes.bfloat16
F8 = mybir.dt.float8e4
F8H = ml_dtypes.float8_e4m3
WS = 16.0      # weight pre-scale shipped from host (W1-side and W2-side)
VP = 1024      # padded categorical vocab (DoubleRow stride alignment)

P = 128
D = 512
H = 256
V = 1000
B = 2048
NC, NN, NB = 32, 16, 16
FC, FN, FB = 4, 2, 2
NF = FC + FN + FB
NBT = 4
BT = 512
NBS = 4
NS = NBT * NBS
LN_EPS = 1e-5
N_CORES = 8

LAST_RESULTS = None


def _build():
    nc = bacc.Bacc("TRN2", target_bir_lowering=False, debug=False,
                   num_devices=N_CORES)
    io = {}

    def din(name, shape, dt=BF16):
        io[name] = nc.dram_tensor(name, shape, dt, kind="ExternalInput").ap()

    din("xt", [NF, P, 4, B], F8)
    din("w1", [P, 4 * 4 * P], F8)
    din("b1", [P, 4], F32)
    din("w1p", [P, NF * 4 * 2 * P], F8)
    din("w1r", [1, NF * 2 * H])
    din("w2c", [P, FC * 2 * VP], F8)
    din("w2n", [P, FN * 2 * D], F8)
    din("w2b", [P, FB * 2 * 2], F8)
    din("browc", [1, FC, VP])
    din("brown", [1, FN, D])
    din("browb", [1, FB, 2])
    din("wct", [FC, P, 2, B], F8)
    din("tgt", [FN, B, D])
    din("maskall", [P, NF, NS], F32)
    din("auxall", [P, NF, NS], F32)
    out = nc.dram_tensor("loss_out", [2 * NF], F32, kind="ExternalOutput").ap()

    with tile.TileContext(nc) as tc:
        with ExitStack() as ctx:
            build_body(ctx, tc, io, out)
    nc.compile()
    return nc


def build_body(ctx, tc, io, out):
    nc = tc.nc

    const = ctx.enter_context(tc.tile_pool(name="const", bufs=1))
    pers = ctx.enter_context(tc.tile_pool(name="pers", bufs=1))
    abuf = ctx.enter_context(tc.tile_pool(name="abuf", bufs=2))
    sbuf2 = ctx.enter_context(tc.tile_pool(name="sbuf2", bufs=2))
    dpool = ctx.enter_context(tc.tile_pool(name="dram", bufs=2, space="DRAM"))
    ps_sh = ctx.enter_context(tc.tile_pool(name="ps_sh", bufs=2, space="PSUM"))
    ps_st = ctx.enter_context(tc.tile_pool(name="ps_st", bufs=1, space="PSUM"))
    ps_h1 = ctx.enter_context(tc.tile_pool(name="ps_h1", bufs=1, space="PSUM"))
    ps_q = ctx.enter_context(tc.tile_pool(name="ps_q", bufs=2, space="PSUM"))

    # ---- constants ----
    ones_st = const.tile([P, 2, 16], F8)
    nc.vector.memset(ones_st, 1.0 / D)
    ones_pk = const.tile([P, 1], BF16)
    nc.vector.memset(ones_pk, 1.0)
    ones_f = const.tile([P, 1], F32)
    nc.vector.memset(ones_f, 1.0)
    eps_t = const.tile([P, 1], F32)
    nc.vector.memset(eps_t, LN_EPS)
    b1_t = const.tile([P, 4], F32)
    nc.sync.dma_start(out=b1_t, in_=io["b1"])
    w1_f = const.tile([P, 4 * 4 * P], F8, name="w1f")
    nc.sync.dma_start(out=w1_f, in_=io["w1"])
    w1_t = w1_f.rearrange("p (a e q) -> p a e q", a=4, e=4, q=P)
    w1p_f = const.tile([P, NF * 4 * 2 * P], F8, name="w1pf")
    w1p = w1p_f.rearrange("p (f a h q) -> p f a h q", f=NF, a=4, h=2, q=P)
    w1r_f = const.tile([1, NF * 2 * H], BF16, name="w1rf")
    w1r = w1r_f.rearrange("o (f a h) -> o f a h", f=NF, a=2, h=H)
    w2c_f = const.tile([P, FC * 2 * VP], F8, name="w2cf")
    w2c = w2c_f.rearrange("p (j h v) -> p j h v", j=FC, h=2, v=VP)
    w2n_f = const.tile([P, FN * 2 * D], F8, name="w2nf")
    w2n = w2n_f.rearrange("p (j h d) -> p j h d", j=FN, h=2, d=D)
    w2b_f = const.tile([P, FB * 2 * 2], F8, name="w2bf")
    w2b = w2b_f.rearrange("p (j h d) -> p j h d", j=FB, h=2, d=2)
    browc = const.tile([1, FC, VP], BF16)
    brown = const.tile([1, FN, D], BF16)
    browb = const.tile([1, FB, 2], BF16)
    maskall = const.tile([P, NF, NS], F32)
    auxall = const.tile([P, NF, NS], F32)
    cebs = [const.tile([P, NS], F32, tag=f"ceb{f}", name=f"ceb{f}")
            for f in range(NF)]

    def emit_A(bt, gate):
        """Phase A+S for batch tile bt: shared layer, LN stats, std rows.
        Returns per-tile state consumed by emit_B."""
        bsl = slice(bt * BT, (bt + 1) * BT)
        st = {"hraws": [], "srows": []}
        var_all = sbuf2.tile([P, NF, NBS], F32, tag="var", name="var_all")
        for f in range(NF):
            xt_t = abuf.tile([P, 4, BT], F8, tag="xt", name="xt_t")
            nc.sync.dma_start(out=xt_t, in_=io["xt"][f][:, :, bsl])
            hraw = pers.tile([P, 4, BT], F8, tag=f"hraw{f}", bufs=2,
                             name="hraw")
            for ec in range(4):
                psh = ps_sh.tile([P, BT], F32, tag="psh", name="psh")
                for dp in range(2):
                    nc.tensor.matmul(
                        psh, w1_t[:, 2 * dp:2 * dp + 2, ec, :],
                        xt_t[:, 2 * dp:2 * dp + 2, :],
                        start=(dp == 0), stop=(dp == 1),
                        perf_mode=mybir.MatmulPerfMode.DoubleRow)
                nc.scalar.activation(hraw[:, ec, :], psh, AF.Gelu,
                                     bias=gate[:, ec:ec + 1],
                                     scale=1.0 / WS)
            h2 = abuf.tile([P, 4, BT], F8, tag="h2", name="h2")
            nc.gpsimd.tensor_mul(h2, hraw, hraw)
            pst = ps_st.tile([1, 2 * BT], F32, tag="pst", name="pst")
            for dp in range(2):
                nc.tensor.matmul(pst[:, 0:BT], ones_st[:, :, 0:1],
                                 hraw[:, 2 * dp:2 * dp + 2, :],
                                 start=(dp == 0), stop=(dp == 1),
                                 perf_mode=mybir.MatmulPerfMode.DoubleRow)
            for dp in range(2):
                nc.tensor.matmul(pst[:, BT:2 * BT], ones_st[:, :, 0:1],
                                 h2[:, 2 * dp:2 * dp + 2, :],
                                 start=(dp == 0), stop=(dp == 1),
                                 perf_mode=mybir.MatmulPerfMode.DoubleRow)
            sr = pers.tile([1, 2 * BT], BF16, tag=f"srow{f}", bufs=2,
                           name="sr")
            nc.vector.tensor_scalar_mul(sr[:, 0:BT], pst[:, 0:BT], 1.0)
            nc.vector.tensor_scalar_mul(sr[:, BT:2 * BT], pst[:, BT:2 * BT],
                                        1.0)
            dstat = dpool.tile([1, 2 * BT], BF16, tag="dstat", name="dstat")
            nc.sync.dma_start(out=dstat, in_=sr)
            colst = sbuf2.tile([P, 2, NBS], BF16, tag="colst", name="colst")
            nc.sync.dma_start(
                out=colst,
                in_=dstat.rearrange("o (q bs p) -> (o p) q bs",
                                    q=2, bs=NBS, p=P))
            musq = sbuf2.tile([P, NBS], F32, tag="musq", name="musq")
            nc.gpsimd.tensor_mul(musq, colst[:, 0, :], colst[:, 0, :])
            nc.gpsimd.tensor_sub(var_all[:, f, :], colst[:, 1, :], musq)
            st["hraws"].append(hraw)
            st["srows"].append(sr)

        # phase S: std + 1/std
        secol = sbuf2.tile([P, NF, NBS], BF16, tag="secol", name="secol")
        nc.scalar.activation(secol, var_all, AF.Sqrt, bias=eps_t[:, 0:1])
        lam = sbuf2.tile([P, NF, NBS], F32, tag="lam", name="lam")
        nc.vector.reciprocal(lam, secol)
        lam256 = sbuf2.tile([P, NF, NBS], F32, tag="lam256", name="lam256")
        nc.gpsimd.tensor_scalar_mul(lam256, lam, 1.0 / (WS * WS))
        lam256n = sbuf2.tile([P, NF, NBS], F32, tag="lam256n", name="lam256n")
        nc.gpsimd.tensor_scalar_mul(lam256n, lam256, -2.0)
        dse = dpool.tile([1, NF, BT], BF16, tag="dse", name="dse")
        nc.sync.dma_start(
            out=dse.rearrange("o f (bs p) -> (o p) f bs", bs=NBS, p=P),
            in_=secol)
        serow = sbuf2.tile([1, NF, BT], BF16, tag="serow", name="serow")
        nc.sync.dma_start(out=serow, in_=dse)
        st["lam256"] = lam256
        st["lam256n"] = lam256n
        st["serow"] = serow
        return st

    def emit_B(bt, st, make_gate):
        """Phase B for batch tile bt: heads, exp, ln, CE assembly."""
        bsl = slice(bt * BT, (bt + 1) * BT)
        csl = slice(bt * NBS, (bt + 1) * NBS)
        lam256 = st["lam256"]
        lam256n = st["lam256n"]
        serow = st["serow"]
        lnall = sbuf2.tile([P, FC + FB, NBS], F32, tag="lnall", name="lnall")
        t1cs = {}
        t1bs = {}
        # interleave feature kinds: numerical/boolean features emit no (or
        # tiny) ACT work, so spreading them between categorical features
        # keeps the Exp stream dense instead of leaving a 15us ACT hole
        # at the tail of every tile
        for f in (0, 4, 1, 6, 2, 5, 3, 7):
            hraw = st["hraws"][f]
            murow = st["srows"][f][0:1, 0:BT]
            serow_f = serow[0:1, f, :]
            psh1 = ps_h1.tile([P, 2, BT], F32, tag="psh1", name="psh1")
            hcT = abuf.tile([P, 2, BT], F8, tag="hcT", name="hcT")
            # the head1->relu chain gates the Exp stream; raise its
            # priority so it preempts the next tile's shared-layer backlog
            # on PE/DVE (DMA triggers stay at natural priority)
            with tc.high_priority():
                for hc in range(2):
                    for dp in range(2):
                        nc.tensor.matmul(
                            psh1[:, hc, :],
                            w1p[:, f, 2 * dp:2 * dp + 2, hc, :],
                            hraw[:, 2 * dp:2 * dp + 2, :],
                            start=(dp == 0), stop=False,
                            perf_mode=mybir.MatmulPerfMode.DoubleRow)
                    nc.tensor.matmul(psh1[:, hc, :],
                                     w1r[0:1, f, 0, hc * P:(hc + 1) * P],
                                     murow, start=False, stop=False)
                    nc.tensor.matmul(psh1[:, hc, :],
                                     w1r[0:1, f, 1, hc * P:(hc + 1) * P],
                                     serow_f, start=False, stop=True)
                for hc in range(2):
                    nc.vector.tensor_scalar_max(hcT[:, hc, :],
                                                psh1[:, hc, :], 0.0)

            if f < FC:
                j = f
                wct_t = abuf.tile([P, 2, BT], F8, tag="wct", name="wct_t")
                nc.sync.dma_start(out=wct_t, in_=io["wct"][j][:, :, bsl])
                prod = abuf.tile([P, 2, BT], BF16, tag="prod", name="prod")
                nc.gpsimd.tensor_mul(prod, hcT, wct_t)
                psqt = ps_sh.tile([P, BT], F32, tag="psh", name="psqt")
                nc.tensor.matmul(psqt[0:1, :], ones_pk, prod[:, 0, :],
                                 start=True, stop=False)
                nc.tensor.matmul(psqt[0:1, :], ones_pk, prod[:, 1, :],
                                 start=False, stop=True)
                qtrow = sbuf2.tile([1, BT], F32, tag="qtrow", name="qtrow")
                nc.vector.tensor_scalar_mul(qtrow, psqt[0:1, :], 1.0)
                dqt = dpool.tile([1, BT], F32, tag="dqt", name="dqt")
                nc.sync.dma_start(out=dqt, in_=qtrow)
                qtcol = sbuf2.tile([P, NBS], F32, tag="qtcol", name="qtcol")
                nc.sync.dma_start(
                    out=qtcol,
                    in_=dqt.rearrange("o (bs p) -> (o p) bs", bs=NBS, p=P))
                ssc = pers.tile([P, NBS, 2], F32, tag=f"ssc{f}", name="ssc")
                for bs in range(NBS):
                    bpart = slice(bs * P, (bs + 1) * P)
                    for vi in range(2):
                        vsl = slice(vi * 512, (vi + 1) * 512)
                        nv = 512 if vi == 0 else V - 512
                        psq = ps_q.tile([P, 512], F32, tag="psq", name="psq")
                        with tc.high_priority():
                            nc.tensor.matmul(
                                psq, hcT[:, :, bpart], w2c[:, j, :, vsl],
                                start=True, stop=False,
                                perf_mode=mybir.MatmulPerfMode.DoubleRow)
                            nc.tensor.matmul(psq, serow_f[0:1, bpart],
                                             browc[0:1, j, vsl],
                                             start=False, stop=True)
                        u = abuf.tile([P, 512], BF16, tag="u", name="u")
                        nc.scalar.activation(u[:, 0:nv], psq[:, 0:nv], AF.Exp,
                                             scale=lam256[:, f, bs:bs + 1],
                                             accum_out=ssc[:, bs, vi:vi + 1])
                t0 = sbuf2.tile([P, NBS], F32, tag="t0", name="t0")
                nc.gpsimd.tensor_mul(t0, qtcol, lam256[:, f, :])
                t1c = pers.tile([P, NBS], F32, tag=f"t1c{f}", name="t1c")
                nc.gpsimd.tensor_add(t1c, t0, auxall[:, f, csl])
                nc.gpsimd.tensor_add(lnall[:, f, :], ssc[:, :, 0],
                                     ssc[:, :, 1])
                t1cs[f] = t1c
            elif f < FC + FN:
                j = f - FC
                tg = abuf.tile([P, NBS, D], BF16, tag="tg", name="tg")
                nc.sync.dma_start(
                    out=tg,
                    in_=io["tgt"][j][bsl].rearrange("(bs p) d -> p bs d",
                                                    bs=NBS, p=P))
                msec = pers.tile([P, NBS], F32, tag=f"msec{f}", name="msec")
                for bs in range(NBS):
                    bpart = slice(bs * P, (bs + 1) * P)
                    psq = ps_q.tile([P, 512], F32, tag="psq", name="psq")
                    with tc.high_priority():
                        nc.tensor.matmul(
                            psq, hcT[:, :, bpart], w2n[:, j, :, :],
                            start=True, stop=False,
                            perf_mode=mybir.MatmulPerfMode.DoubleRow)
                        nc.tensor.matmul(psq, serow_f[0:1, bpart],
                                         brown[0:1, j, :],
                                         start=False, stop=True)
                    diff = abuf.tile([P, D], F32, tag="diff", name="diff")
                    nc.vector.scalar_tensor_tensor(
                        out=diff, in0=psq, scalar=lam256[:, f, bs:bs + 1],
                        in1=tg[:, bs, :], op0=OP.mult, op1=OP.subtract)
                    sq = abuf.tile([P, D], BF16, tag="sq", name="sq")
                    nc.vector.scalar_tensor_tensor(
                        out=sq, in0=diff, scalar=1.0, in1=diff,
                        op0=OP.bypass, op1=OP.mult,
                        accum_out=msec[:, bs:bs + 1])
                nc.vector.scalar_tensor_tensor(
                    out=cebs[f][:, csl], in0=msec, scalar=1.0 / D,
                    in1=maskall[:, f, csl], op0=OP.mult, op1=OP.mult)
            else:
                j = f - FC - FN
                psq = ps_q.tile([P, 512], F32, tag="psq", name="psq")
                pb = psq[:, 0:2 * NBS].rearrange("p (bs two) -> p bs two",
                                                 two=2)
                with tc.high_priority():
                    for bs in range(NBS):
                        bpart = slice(bs * P, (bs + 1) * P)
                        nc.tensor.matmul(pb[:, bs, :], hcT[:, :, bpart],
                                         w2b[:, j, :, :], start=True,
                                         stop=False,
                                         perf_mode=mybir.MatmulPerfMode
                                         .DoubleRow)
                        nc.tensor.matmul(pb[:, bs, :], serow_f[0:1, bpart],
                                         browb[0:1, j, :], start=False,
                                         stop=True)
                zb = pers.tile([P, NBS, 2], F32, tag=f"zb{f}", name="zb")
                for bs in range(NBS):
                    nc.vector.tensor_scalar_mul(
                        zb[:, bs, :], pb[:, bs, :], lam256[:, f, bs:bs + 1])
                u2 = sbuf2.tile([P, NBS, 2], F32, tag="u2", name="u2")
                nc.scalar.activation(u2, zb, AF.Exp)
                nc.gpsimd.tensor_add(lnall[:, FC + j, :],
                                     u2[:, :, 0], u2[:, :, 1])
                dlt = sbuf2.tile([P, NBS], F32, tag="dlt", name="dlt")
                nc.gpsimd.tensor_sub(dlt, zb[:, :, 1], zb[:, :, 0])
                ta = sbuf2.tile([P, NBS], F32, tag="ta", name="ta")
                nc.gpsimd.tensor_mul(ta, dlt, auxall[:, f, csl])
                t1b = pers.tile([P, NBS], F32, tag=f"t1b{f}", name="t1b")
                nc.gpsimd.tensor_add(t1b, ta, zb[:, :, 0])
                t1bs[f] = t1b

        # phase B2: one Ln + CE assembly
        lns_all = sbuf2.tile([P, FC + FB, NBS], F32, tag="lns", name="lns")
        nc.scalar.activation(lns_all, lnall, AF.Ln)
        for f in range(NF):
            if FC <= f < FC + FN:
                continue
            k = f if f < FC else f - FN
            t1 = t1cs[f] if f < FC else t1bs[f]
            tmp = sbuf2.tile([P, NBS], F32, tag="tmp", name="tmp")
            nc.gpsimd.tensor_sub(tmp, lns_all[:, k, :], t1)
            nc.gpsimd.tensor_mul(cebs[f][:, csl], tmp, maskall[:, f, csl])
        if not make_gate:
            return None
        # ACT-order gate: tile bt+2's Gelus read the gate, which
        # (bypass-)depends on this tile's Ln output, so the scheduler keeps
        # at most one Gelu phase ahead of the Exp/Ln stream (table thrash).
        b1g = sbuf2.tile([P, 4], F32, tag="b1gate", name="b1g")
        with tc.high_priority():
            nc.vector.scalar_tensor_tensor(
                out=b1g, in0=b1_t, scalar=1.0, in1=lns_all[:, 0, :],
                op0=OP.mult, op1=OP.bypass)
        return b1g

    # 1-deep software pipeline: A(k+1) is emitted before B(k) so the LN
    # stats round trip of tile k+1 hides under tile k's head phase.
    states = {0: emit_A(0, b1_t)}
    # big weight/const loads queued after tile 0's xt loads so the first
    # Gelus aren't starved behind them on the DMA engines
    nc.sync.dma_start(out=w1p_f, in_=io["w1p"])
    nc.sync.dma_start(out=w2c_f, in_=io["w2c"])
    nc.sync.dma_start(out=w2n_f, in_=io["w2n"])
    nc.sync.dma_start(out=w2b_f, in_=io["w2b"])
    nc.sync.dma_start(out=browc, in_=io["browc"])
    nc.sync.dma_start(out=brown, in_=io["brown"])
    nc.sync.dma_start(out=browb, in_=io["browb"])
    nc.sync.dma_start(out=maskall, in_=io["maskall"])
    nc.sync.dma_start(out=auxall, in_=io["auxall"])
    nc.sync.dma_start(out=w1r_f, in_=io["w1r"])
    for k in range(NBT):
        if k + 1 < NBT:
            gate = b1_t if k < 1 else gates[k - 1]
            states[k + 1] = emit_A(k + 1, gate)
        g = emit_B(k, states.pop(k), make_gate=(k + 2 < NBT))
        if k == 0:
            gates = {}
        if g is not None:
            gates[k] = g

    # ---------------- final reduction --------------------------------
    acc2 = const.tile([P, 2 * NF], F32)
    for f in range(NF):
        nc.vector.reduce_sum(acc2[:, f:f + 1], cebs[f], axis=AX.X)
        nc.vector.reduce_sum(acc2[:, NF + f:NF + f + 1], maskall[:, f, :],
                             axis=AX.X)
    pso = ps_q.tile([P, 512], F32, tag="psq")
    nc.tensor.matmul(pso[0:1, 0:2 * NF], ones_f, acc2, start=True, stop=True)
    outc = const.tile([1, 2 * NF], F32)
    nc.vector.tensor_scalar_mul(outc, pso[0:1, 0:2 * NF], 1.0)
    nc.sync.dma_start(out=out.rearrange("(o c) -> o c", o=1), in_=outc)


_NC_CACHE = None


def _get_nc():
    global _NC_CACHE
    if _NC_CACHE is None:
        _NC_CACHE = _build()
    return _NC_CACHE


def _prep_core(i, seq, targets, mask_f, cat_t, bool_t, w):
    """Build the in_map for core i. All layout / slicing, no data math."""
    cg = list(range(4 * i, 4 * i + 4))
    ng = [2 * i, 2 * i + 1]
    bg = [2 * i, 2 * i + 1]
    feats = cg + [NC + g for g in ng] + [NC + NN + g for g in bg]

    xf = seq[:, feats, :]                                   # [B, NF, D]
    xt = (xf.transpose(1, 2, 0).reshape(NF, 4, P, B)
          .transpose(0, 2, 1, 3))                           # [NF, P, 4, B]

    w1p_all = np.concatenate([w["wc1p"][cg], w["wn1p"][ng], w["wb1p"][bg]])
    w1p = (w1p_all.reshape(NF, 4, P, 2, P)
           .transpose(2, 0, 1, 3, 4))                       # [P, NF, 4, 2, P]
    w1r = np.concatenate([w["wc1r"][cg], w["wn1r"][ng], w["wb1r"][bg]])

    wc2p = np.zeros((FC, H, VP), np.float32)
    wc2p[:, :, :V] = w["Wc2"][cg]
    w2c = wc2p.reshape(FC, 2, P, VP).transpose(2, 0, 1, 3)
    w2n = w["Wn2"][ng].reshape(FN, 2, P, D).transpose(2, 0, 1, 3)
    w2b = w["Wb2"][bg].reshape(FB, 2, P, 2).transpose(2, 0, 1, 3)

    wct = np.stack([w["Wc2"][g][:, cat_t[:, g]].reshape(2, P, B)
                    .transpose(1, 0, 2) for g in cg])       # [FC, P, 2, B]

    tgt = targets[:, [NC + g for g in ng], :].transpose(1, 0, 2)  # [FN, B, D]

    maskall = (mask_f[:, feats].reshape(NS, P, NF)
               .transpose(1, 2, 0))                         # [P, NF, NS]
    auxall = np.zeros((P, NF, NS), np.float32)
    for k, g in enumerate(cg):
        auxall[:, k, :] = w["bc2"][g][cat_t[:, g]].reshape(NS, P).T
    for k, g in enumerate(ng):
        tsq = (targets[:, NC + g, :].astype(np.float64) ** 2).sum(-1)
        auxall[:, FC + k, :] = tsq.astype(np.float32).reshape(NS, P).T
    for k, g in enumerate(bg):
        auxall[:, FC + FN + k, :] = bool_t[:, g].astype(
            np.float32).reshape(NS, P).T

    bc2p = np.zeros((FC, VP), np.float32)
    bc2p[:, :V] = w["bc2"][cg]
    m8 = {
        "xt": xt,
        "w1": w["w1h"].reshape(P, -1) * WS,
        "w1p": w1p.reshape(P, -1) * WS,
        "w2c": w2c.reshape(P, -1) * WS,
        "w2n": w2n.reshape(P, -1) * WS,
        "w2b": w2b.reshape(P, -1) * WS,
        "wct": wct * WS,
    }
    m16 = {
        "w1r": w1r.reshape(1, -1) * WS,
        "browc": bc2p[None] * (WS * WS),
        "brown": w["bn2"][ng][None] * (WS * WS),
        "browb": w["bb2"][bg][None] * (WS * WS),
        "tgt": tgt,
    }
    m = {k: np.ascontiguousarray(v.astype(F8H)) for k, v in m8.items()}
    m.update({k: np.ascontiguousarray(v.astype(BF)) for k, v in m16.items()})
    m["b1"] = np.ascontiguousarray(w["b1h"], np.float32)
    m["maskall"] = np.ascontiguousarray(maskall, np.float32)
    m["auxall"] = np.ascontiguousarray(auxall, np.float32)
    return m


def prepare_in_maps(inputs):
    seq = np.asarray(inputs["sequence_embeddings"], np.float32)
    targets = np.asarray(inputs["targets"], np.float32)
    mask_f = np.asarray(inputs["target_mask"]).astype(np.float32)
    cat_t = np.asarray(inputs["cat_targets"]).astype(np.int64)
    bool_t = np.asarray(inputs["bool_targets"]).astype(np.int64)

    ln_g = np.asarray(inputs["ln_g"], np.float64)
    ln_b = np.asarray(inputs["ln_b"], np.float64)

    def fold(w1, b1):
        w1 = np.asarray(w1, np.float64)
        b1 = np.asarray(b1, np.float64)
        wp = ln_g[None, :, None] * w1                    # [F, D, H]
        bp = b1 + np.einsum("d,fdh->fh", ln_b, w1)       # [F, H]
        rows = np.stack([-wp.sum(1), bp], axis=1)        # [F, 2, H]
        return wp.astype(np.float32), rows.astype(np.float32)

    W1 = np.asarray(inputs["W1"], np.float32)
    b1 = np.asarray(inputs["b1"], np.float32)
    w = {
        "w1h": W1.reshape(4, P, 4, P).transpose(1, 0, 2, 3),
        "b1h": b1.reshape(4, P).T,
        "Wc2": np.asarray(inputs["Wc2"], np.float32),
        "bc2": np.asarray(inputs["bc2"], np.float32),
        "Wn2": np.asarray(inputs["Wn2"], np.float32),
        "bn2": np.asarray(inputs["bn2"], np.float32),
        "Wb2": np.asarray(inputs["Wb2"], np.float32),
        "bb2": np.asarray(inputs["bb2"], np.float32),
    }
    w["wc1p"], w["wc1r"] = fold(inputs["Wc1"], inputs["bc1"])
    w["wn1p"], w["wn1r"] = fold(inputs["Wn1"], inputs["bn1"])
    w["wb1p"], w["wb1r"] = fold(inputs["Wb1"], inputs["bb1"])

    return [_prep_core(i, seq, targets, mask_f, cat_t, bool_t, w)
            for i in range(N_CORES)]


def combine(per_core_outs):
    total = 0.0
    for r in per_core_outs:
        r = np.asarray(r, np.float64)
        s, c = r[:NF], r[NF:]
        total += np.where(c > 0, s / np.maximum(c, 1.0), 0.0).sum()
    return np.float32(total)


def kernel(**inputs):
    global LAST_RESULTS
    in_maps = prepare_in_maps(inputs)
    nc = _get_nc()
    res = run_bass_kernel_spmd(nc, in_maps, core_ids=list(range(N_CORES)))
    LAST_RESULTS = res
    return combine([res.results[i]["loss_out"] for i in range(N_CORES)])
